# revision 10
# baseline (speedup 1.0000x reference)
"""Trainium2 Bass kernel for nn_BasicRecurrentEntityEncoder.

Math (per paragraph b, per step t, state h [K, D]):
    g   = sigmoid(s . (h + keys))            per entity gate
    ht  = relu(h U + keys V + s W)
    upd = h + g * ht
    h'  = upd / ||upd||_2   where active (mask), else h unchanged

Device mapping (8 cores, pure data parallel, 32 paragraphs/core):
  * rows r = (paragraph, entity) = 2048 rows/core, 16 tiles of 128.
  * masked steps are identity -> host compresses each paragraph's sequence
    to its active steps; loop runs T = max active count (~S/2) steps with a
    per-(row, step) pad mask folding the tail.
  * state: upd (fp32, unnormalized), delta (per-row 1/norm), h_bf = delta*upd
    (bf16, natural) + hT_bf (bf16, transposed shadow via DMA-transpose) feeding
    the PE.
  * per step PE passes (all bf16, fp32 PSUM accum):
      A:  pre  += h @ U          (stationary = hT tile, rhs U)
      Ag: gdot  = h @ s_cols     (2 cols per tile -> gate dots)
      B1: pre  += keys@V         (stationary = I, rhs = kv tile)
      B2: pre  += s@W broadcast  (stationary = I, rhs = swrep tile from DMA)
  * custom DVE op: upd' = upd*delta + relu(pre*gamma)  per tile.
  * n2 via bn_stats; delta' = exp(-0.5*ln(max(n2,1e-12))) (one ACT table set);
    gamma = pad*sigmoid(glog) via exp + reciprocal.
"""

import os
import sys

sys.path.insert(0, "/opt/trn_rl_repo")

import numpy as np
import ml_dtypes
from contextlib import ExitStack

import concourse.bass as bass
import concourse.bacc as bacc
import concourse.mybir as mybir
from concourse.tile import TileContext

BF16 = mybir.dt.bfloat16
F32 = mybir.dt.float32
AF = mybir.ActivationFunctionType
ALU = mybir.AluOpType

B, S, K, D = 256, 256, 64, 128
NCORES = 8
BL = B // NCORES  # 32 local paragraphs
NT = 16  # row tiles per core (2048 rows / 128)


# ------------------------------------------------------------------ custom op
def get_update_op():
    """out = in0*s0 + relu(in1*s1)   (upd*delta + relu(pre*gamma))."""
    from concourse import dve_ops as dv
    from concourse.dve_spec import Spec, Src0, Src1, C0, C1, relu, lower, _has_src1
    from concourse.dve_uop import DveOpSpec

    name = "SCALE_ADD_RELU_SCALED_ANT"
    for o in dv.OPS:
        if o.name == name:
            return o

    def _ref(in0, in1, s0, s1, imm2):
        x = in1.astype(np.float32) * s1
        x = np.nan_to_num(x, nan=0.0, posinf=np.inf, neginf=-np.inf)
        return in0.astype(np.float32) * s0 + np.maximum(x, 0.0)

    spec = Spec(body=Src0 * C0 + relu(Src1 * C1), reference=_ref)
    row = max(dv._SUB_OPCODE_FOR_NAME.values()) + 1
    assert row < 0x20, "no free custom-DVE opcode rows"
    dv._SUB_OPCODE_FOR_NAME[name] = row
    shas = {}
    for ver in ("v3", "v4"):
        try:
            uops = lower(spec, ver=ver)
            shas[ver] = DveOpSpec(
                name=name, opcode=row, uops=uops, rd1_en=_has_src1(spec)
            ).sha(ver)
        except Exception:
            pass
    assert "v3" in shas, "custom op failed to lower for TRN2"
    op = dv.DveOp(name, spec, subdim=False, uops_sha=shas)
    dv.OPS.append(op)
    dv.CUSTOM_DVE_SPECS[name] = spec
    return op


# ------------------------------------------------------------------ program
def build_program(T, sim=False):
    """Emit the full per-core Bass program. Returns nc."""
    op_upd = get_update_op()
    nc = bacc.Bacc("TRN2", target_bir_lowering=False)

    # ---- I/O (packed blobs: few DMA queues -> few sem waits downstream)
    NFB = NT * 128 + T * BL + 4 * 128  # keysT | sT | U | V | W | ident
    blob_in = nc.dram_tensor("blob16", [128, NFB], BF16, kind="ExternalInput")
    pad_in = nc.dram_tensor("pad", [128, T, NT], F32, kind="ExternalInput")
    hfin_out = nc.dram_tensor("hfin", [128, NT, 128], F32, kind="ExternalOutput")
    # sW table staged in DRAM for the per-step broadcast
    sw_dram = nc.dram_tensor("sw_stage", [T, BL, 128], BF16, kind="Internal")

    TC = (T * BL + 127) // 128  # 128-col chunks of the (t, j) axis

    with ExitStack() as ctx:
        tc = ctx.enter_context(TileContext(nc))
        ec = ctx.enter_context

        # ---- persistent SBUF
        blob_sb = ec(nc.sbuf_tensor("blob_sb", [128, NFB], BF16))
        o = 0
        keysT_sb = blob_sb[:, o : o + NT * 128].rearrange(
            "p (i e) -> p i e", i=NT
        ); o += NT * 128
        sT_sb = blob_sb[:, o : o + T * BL].rearrange(
            "p (t j) -> p t j", t=T
        ); o += T * BL
        U_sb = blob_sb[:, o : o + 128]; o += 128
        V_sb = blob_sb[:, o : o + 128]; o += 128
        W_sb = blob_sb[:, o : o + 128]; o += 128
        I_sb = blob_sb[:, o : o + 128]; o += 128
        pad_sb = ec(nc.sbuf_tensor("pad_sb", [128, T, NT], F32))
        kv_sb = ec(nc.sbuf_tensor("kv_sb", [128, NT, 128], BF16))
        SK_sb = ec(nc.sbuf_tensor("SK_sb", [128, T, NT], F32))
        swT_sb = ec(nc.sbuf_tensor("swT_sb", [128, TC * 128], BF16))
        swch_sb = ec(nc.sbuf_tensor("swch_sb", [128, TC, 128], BF16))
        swrep_sb = ec(nc.sbuf_tensor("swrep_sb", [128, 2, NT, 128], BF16))
        h_sb = ec(nc.sbuf_tensor("h_sb", [128, NT, 128], BF16))
        hT_sb = ec(nc.sbuf_tensor("hT_sb", [128, NT, 128], BF16))
        upd_sb = ec(nc.sbuf_tensor("upd_sb", [128, NT, 128], F32))
        dl_sb = ec(nc.sbuf_tensor("dl_sb", [128, NT], F32))
        hfin_sb = ec(nc.sbuf_tensor("hfin_sb", [128, NT, 128], F32))
        # small transients
        gsel_sb = ec(nc.sbuf_tensor("gsel_sb", [128, NT], F32))
        glog_sb = ec(nc.sbuf_tensor("glog_sb", [128, NT], F32))
        ex_sb = ec(nc.sbuf_tensor("ex_sb", [128, NT], F32))
        den_sb = ec(nc.sbuf_tensor("den_sb", [128, NT], F32))
        gam0_sb = ec(nc.sbuf_tensor("gam0_sb", [128, NT], F32))
        gam_sb = ec(nc.sbuf_tensor("gam_sb", [128, NT], F32))
        sq_sb = ec(nc.sbuf_tensor("sq_sb", [128, NT, 128], F32))
        n2_sb = ec(nc.sbuf_tensor("n2_sb", [128, NT], F32))
        n2c_sb = ec(nc.sbuf_tensor("n2c_sb", [128, NT], F32))
        dlr_sb = ec(nc.sbuf_tensor("dlr_sb", [128, NT], F32))
        dlm_sb = ec(nc.sbuf_tensor("dlm_sb", [128, NT], F32))
        # psum
        ps_main = ec(nc.psum_tensor("ps_main", [128, NT * 128], F32))
        ps_g = ec(nc.psum_tensor("ps_g", [128, 2 * NT], F32))
        ps_aux = ec(nc.psum_tensor("ps_aux", [128, 512], F32))

        sync = nc.sync
        vec = nc.vector
        act = nc.scalar
        pe = nc.tensor

        # ================= setup =================
        sync.dma_start(blob_sb[:], blob_in[:], max_dma_last_dim=65024)
        sync.dma_start(pad_sb[:], pad_in[:], max_dma_last_dim=65024)

        vec.memset(h_sb[:], 0)
        vec.memset(hT_sb[:], 0)
        vec.memset(upd_sb[:], 0.0)
        vec.memset(dl_sb[:], 1.0)

        # kv = keys @ V   (natural tiles)
        for i in range(NT):
            pe.matmul(
                ps_main[:, i * 128 : (i + 1) * 128],
                lhsT=keysT_sb[:, i, :],
                rhs=V_sb,
                start=(i % 4 == 0),
                stop=(i % 4 == 3),
            )
        for c in range(4):
            vec.tensor_copy(
                kv_sb[:, 4 * c : 4 * (c + 1), :],
                ps_main[:, 512 * c : 512 * (c + 1)],
            )

        # SK[r, t] = s_{b(r), t} . keys_r
        for i in range(NT):
            for j in range(2):
                pe.matmul(
                    ps_aux[:, 0:T],
                    lhsT=keysT_sb[:, i, :],
                    rhs=sT_sb[:, :, 2 * i + j],
                    start=True,
                    stop=True,
                )
                half = slice(0, 64) if j == 0 else slice(64, 128)
                vec.tensor_copy(SK_sb[half, :, i], ps_aux[half, 0:T])

        # sW staging: sWT = W^T @ s  -> chunked transpose -> DRAM [T, BL, 128]
        vec.memset(swT_sb[:], 0)
        for c in range(TC):
            lo = c * 128
            n = min(128, T * BL - lo)
            cols = sT_sb.rearrange("d t j -> d (t j)")[:, lo : lo + n]
            pe.matmul(
                ps_aux[:, 0:n],
                lhsT=W_sb,
                rhs=cols,
                start=True,
                stop=True,
            )
            vec.tensor_copy(swT_sb[:, lo : lo + n], ps_aux[:, 0:n])
        sync.dma_start_transpose(swch_sb[:], swT_sb[:])
        # swch[32*(t%4)+j, t//4, e] = sW[t, j, e]  ->  sw_dram[t, j, e]
        for t4 in range(4):
            n_c = (T - t4 + 3) // 4  # chunks c with t = 4c + t4 < T
            if n_c <= 0:
                continue
            src = swch_sb[32 * t4 : 32 * (t4 + 1), 0:n_c, :]
            hi = t4 + 4 * (n_c - 1) + 1
            dst = sw_dram[t4:hi:4, :, :].rearrange("c j e -> j c e")
            sync.dma_start(dst, src)

        def prefetch_sw(t):
            slot = t % 2
            for a in range(2):
                va = sw_dram[t, a : BL : 2, :].rearrange("i (q e) -> q i e", q=1)
                vab = va.broadcast_to([64, NT, 128])
                sync.dma_start(swrep_sb[64 * a : 64 * (a + 1), slot, :, :], vab)

        prefetch_sw(0)

        # ================= time loop =================
        for t in range(T):
            slot = t % 2
            # ---- PE: pre = h U + kv + swrep ; gate dots
            for i in range(NT):
                pe.matmul(
                    ps_main[:, i * 128 : (i + 1) * 128],
                    lhsT=hT_sb[:, i, :],
                    rhs=U_sb,
                    start=(i % 4 == 0),
                    stop=False,
                )
            for i in range(NT):
                pe.matmul(
                    ps_g[:, 2 * i : 2 * i + 2],
                    lhsT=hT_sb[:, i, :],
                    rhs=sT_sb[:, t, 2 * i : 2 * i + 2],
                    start=(i == 0),
                    stop=(i == NT - 1),
                )
            for i in range(NT):
                pe.matmul(
                    ps_main[:, i * 128 : (i + 1) * 128],
                    lhsT=I_sb,
                    rhs=kv_sb[:, i, :],
                    start=False,
                    stop=False,
                )
            for i in range(NT):
                pe.matmul(
                    ps_main[:, i * 128 : (i + 1) * 128],
                    lhsT=I_sb,
                    rhs=swrep_sb[:, slot, i, :],
                    start=False,
                    stop=(i % 4 == 3),
                )
            if t + 1 < T:
                prefetch_sw(t + 1)

            # ---- gate: gamma = pad * sigmoid(gdot + SK)
            vec.tensor_copy(gsel_sb[0:64, :], ps_g[0:64, 0 : 2 * NT : 2])
            vec.tensor_copy(gsel_sb[64:128, :], ps_g[64:128, 1 : 2 * NT : 2])
            vec.tensor_tensor(
                glog_sb[:], gsel_sb[:], SK_sb[:, t, :], op=ALU.add
            )
            act.activation(ex_sb[:], glog_sb[:], AF.Exp, scale=-1.0)
            vec.tensor_scalar_add(den_sb[:], ex_sb[:], 1.0)
            vec.reciprocal(gam0_sb[:], den_sb[:])
            vec.tensor_tensor(
                gam_sb[:], gam0_sb[:], pad_sb[:, t, :], op=ALU.mult
            )

            # ---- state update: upd = upd*delta + relu(pre*gamma)
            for i in range(NT):
                vec._custom_dve(
                    op_upd,
                    out=upd_sb[:, i, :],
                    in0=upd_sb[:, i, :],
                    in1=ps_main[:, i * 128 : (i + 1) * 128],
                    s0=dl_sb[:, i : i + 1],
                    s1=gam_sb[:, i : i + 1],
                )

            # ---- n2 = ||upd||^2 per row (ACT square, DVE reduce)
            act.activation(sq_sb[:], upd_sb[:], AF.Square)
            vec.tensor_reduce(
                n2_sb[:], sq_sb[:], axis=mybir.AxisListType.X, op=ALU.add
            )

            # ---- delta = 1 + pad*(rsqrt(n2) - 1)
            vec.tensor_scalar_max(n2c_sb[:], n2_sb[:], 1e-12)
            act.activation(dlr_sb[:], n2c_sb[:], AF.Ln)
            act.activation(dlr_sb[:], dlr_sb[:], AF.Exp, scale=-0.5)
            vec.scalar_tensor_tensor(
                dlm_sb[:],
                in0=dlr_sb[:],
                scalar=1.0,
                in1=pad_sb[:, t, :],
                op0=ALU.subtract,
                op1=ALU.mult,
            )
            vec.tensor_scalar_add(dl_sb[:], dlm_sb[:], 1.0)

            # ---- h_bf = delta * upd (bf16) ; hT shadow via DMA transpose
            for i in range(NT):
                act.activation(
                    h_sb[:, i, :],
                    upd_sb[:, i, :],
                    AF.Copy,
                    scale=dl_sb[:, i : i + 1],
                )
            sync.dma_start_transpose(
                hT_sb[:], h_sb[:].rearrange("p a b -> p (a b)")
            )

        # ================= output =================
        for i in range(NT):
            act.activation(
                hfin_sb[:, i, :],
                upd_sb[:, i, :],
                AF.Copy,
                scale=dl_sb[:, i : i + 1],
            )
        sync.dma_start(hfin_out[:], hfin_sb[:])

    nc.compile()
    return nc


# ------------------------------------------------------------------ host prep
def prepare_inputs(encoded_sents, mask, keys, U, V, W):
    """Build per-core input maps + metadata. Returns (T, in_maps)."""
    es = np.asarray(encoded_sents, dtype=np.float32)
    mk = np.asarray(mask)
    ks = np.asarray(keys, dtype=np.float32)

    nb = mk.sum(axis=1).astype(np.int64)  # active counts per paragraph
    T = int(nb.max()) if nb.max() > 0 else 1

    bf = ml_dtypes.bfloat16
    U_b = np.asarray(U, dtype=np.float32).astype(bf)
    V_b = np.asarray(V, dtype=np.float32).astype(bf)
    W_b = np.asarray(W, dtype=np.float32).astype(bf)
    ident = np.eye(128, dtype=np.float32).astype(bf)

    in_maps = []
    for c in range(NCORES):
        bs = np.arange(BL) + BL * c  # global paragraph ids
        s_comp = np.zeros((BL, T, D), np.float32)
        padm = np.zeros((BL, T), np.float32)
        for j, b in enumerate(bs):
            idx = np.nonzero(mk[b])[0]
            n = len(idx)
            if n:
                s_comp[j, :n] = es[b, idx]
                padm[j, :n] = 1.0

        # sT[d, t, j]
        sT = np.ascontiguousarray(s_comp.transpose(2, 1, 0)).astype(bf)

        # keysT[d, i, q] = keys[b(i,q), k(q), d];  b_loc = 2i + (q>=64), k = q%64
        kk = ks[bs]  # [BL, K, D]
        q = np.arange(128)
        i_idx = np.arange(NT)
        b_loc = 2 * i_idx[None, :] + (q[:, None] >= 64)  # [128, NT]
        k_of_q = q % 64
        keysT = np.ascontiguousarray(
            kk[b_loc, k_of_q[:, None], :].transpose(2, 1, 0)
        ).astype(bf)
        # keysT now [D, NT, 128]

        # pad[p, t, i] = padm[b_loc(p, i), t]
        padf = np.ascontiguousarray(
            padm[b_loc, :].transpose(0, 2, 1)
        ).astype(np.float32)
        # padf [128, T, NT]

        blob = np.concatenate(
            [
                keysT.reshape(D, NT * 128),
                sT.reshape(D, T * BL),
                U_b,
                V_b,
                W_b,
                ident,
            ],
            axis=1,
        ).astype(bf)
        in_maps.append({"blob16": np.ascontiguousarray(blob), "pad": padf})
    return T, in_maps


def gather_output(results):
    """results: list of dicts with 'hfin' [128, NT, 128] per core -> [B, K, D]."""
    out = np.zeros((B, K, D), np.float32)
    for c in range(NCORES):
        h = results[c]["hfin"]  # [128, NT, 128]
        for b_loc in range(BL):
            i, a = b_loc // 2, b_loc % 2
            out[BL * c + b_loc] = h[64 * a : 64 * a + 64, i, :]
    return out


# ------------------------------------------------------------------ entry
def kernel(encoded_sents, mask, keys, U, V, W):
    from concourse.bass_utils import run_bass_kernel_spmd

    T, in_maps = prepare_inputs(encoded_sents, mask, keys, U, V, W)
    nc = build_program(T)
    res = run_bass_kernel_spmd(nc, in_maps, core_ids=list(range(NCORES)))
    return gather_output(res.results)


# ------------------------------------------------------------------ sim check
def _sim_check():
    """CoreSim single-core run on truncated data vs numpy emulation."""
    from concourse import bass_interp
    import jax

    sys.path.insert(0, os.path.dirname(os.path.abspath(__file__)))
    import reference

    inputs = {k: np.asarray(v) for k, v in reference.setup_inputs().items()}
    # truncate so the sim is fast: keep only first 6 active steps per paragraph
    mask = inputs["mask"].copy()
    for b in range(B):
        idx = np.nonzero(mask[b])[0]
        mask[b, idx[6:]] = False
    inputs["mask"] = mask

    ref = np.asarray(
        reference.reference(
            inputs["encoded_sents"],
            mask,
            inputs["keys"],
            inputs["U"],
            inputs["V"],
            inputs["W"],
        )
    )

    T, in_maps = prepare_inputs(
        inputs["encoded_sents"], mask, inputs["keys"],
        inputs["U"], inputs["V"], inputs["W"],
    )
    print(f"sim T={T}")
    nc = build_program(T, sim=True)
    core = 0
    sim = bass_interp.CoreSim(nc)
    for k, v in in_maps[core].items():
        sim.tensor(k)[:] = v
    sim.simulate()
    got = gather_output([{"hfin": np.array(sim.tensor("hfin"))}] * NCORES)

    g0 = got[:BL]
    r0 = ref[:BL]
    denom = np.abs(r0).max()
    err = np.abs(g0 - r0).max() / denom
    rel = np.linalg.norm(g0 - r0) / np.linalg.norm(r0)
    print(f"sim core0: absmax-rel {err:.3e}  l2-rel {rel:.3e}")
    return err


if __name__ == "__main__":
    _sim_check()
